# revision 23
# baseline (speedup 1.0000x reference)
"""CrossFrameAttention Trainium2 kernel.

Reference computation (B=16, S=6, E=256, heads=8, spatial 64x64):
  g    = mean_{h,w}(frame_features) + temporal_pos          [B,S,E]
  q/k/v= g @ W{q,k,v}.T + b                                 [B,S,E]
  attn = causal softmax(q k^T / sqrt(32)) per head          [B,8,S,S]
  out_g= (attn @ v) @ Wo.T + bo                             [B,S,E]
  cross= frame_features + out_g[..., None, None]
  returns (cross, attn)

Strategy: pure batch data-parallelism, 2 batches per core on 8 cores.
Single-pass streaming per (batch, frame): DMA the 4 MB frame slab into
SBUF, reduce it for the spatial mean, run the (tiny) incremental causal
attention row for that frame on-chip, broadcast-add out_g onto the slab
still resident in SBUF, DMA it out.  The 402 MB input is read exactly
once and written exactly once -> HBM roofline ~100 MB/core.

Channel layout on chip: e = j*128 + p (j in {0,1} selects the half,
p is the SBUF partition).  Head h = j*4 + p//32.

kernel() takes FULL inputs and returns the FULL (cross, attn) outputs.
"""

import math
import os

import numpy as np

B, S, E, HWD = 16, 6, 256, 4096
NH, HD = 8, 32
NCORES = 8
BPC = B // NCORES  # batches per core

_CACHE = {}


def _build_nc(repeat=1, probe=None):
    import concourse.bass as bass
    import concourse.tile as tile
    from concourse import mybir

    f32 = mybir.dt.float32
    AFT = mybir.ActivationFunctionType
    AX = mybir.AxisListType

    nc = bass.Bass(trn_type="TRN2")

    ff = nc.dram_tensor("ff", [BPC, S, E, HWD], f32, kind="ExternalInput")
    # all small constants packed into one tensor -> one DMA -> one sem lane:
    # cols 0:2048   = W{q,k,v,o}.T halves, block (w*2+j)*256 : +256
    # cols 2048:2060 = temporal_pos cols (s*2+j)
    # cols 2060:2068 = biases cols (w*2+j)
    # cols 2068:2196 = head-expand matrix er on partitions 0:4
    CW, CTP, CBC, CER = 0, 2048, 2060, 2068
    CTOT = 2196
    consts = nc.dram_tensor("consts", [128, CTOT], f32, kind="ExternalInput")
    cross = nc.dram_tensor("cross", [BPC, S, E, HWD], f32, kind="ExternalOutput")
    attn = nc.dram_tensor("attn", [BPC, NH, S, S], f32, kind="ExternalOutput")

    SCALE = 1.0 / math.sqrt(HD)

    with tile.TileContext(nc) as tc:
        with tc.tile_pool(name="const", bufs=1) as const, \
             tc.tile_pool(name="slabs", bufs=4) as slabs, \
             tc.tile_pool(name="small", bufs=4) as small, \
             tc.tile_pool(name="cache", bufs=2) as cache, \
             tc.tile_pool(name="psA", bufs=2, space="PSUM") as psA, \
             tc.tile_pool(name="psS", bufs=2, space="PSUM") as psS, \
             tc.tile_pool(name="psE", bufs=2, space="PSUM") as psE, \
             tc.tile_pool(name="psO", bufs=2, space="PSUM") as psO:

            # --- constants: one DMA ---
            cs = const.tile([128, CTOT], f32, tag="consts")
            nc.sync.dma_start(out=cs, in_=consts[:])
            wsb = {}
            for w, nm in enumerate(("q", "k", "v", "o")):
                for j in range(2):
                    base = CW + (w * 2 + j) * 256
                    wsb[nm, j] = cs[:, base:base + 256]
            tp_sb = cs[:, CTP:CTP + 12].rearrange("p (s j) -> p s j", s=S)
            bc_sb = cs[:, CBC:CBC + 8]
            er_sb = cs[0:4, CER:CER + 128]

            # attention probabilities accumulator [r, (b, s, j, t)]
            attn_acc = const.tile([4, BPC * S * 2 * S], f32, tag="attn_acc")
            nc.vector.memset(attn_acc, 0.0)
            # block-diagonal q (per j half): [p, r] nonzero only when p//32==r
            qblk = []
            for j in range(2):
                qb = const.tile([128, 4], f32, tag=f"qblk{j}")
                nc.vector.memset(qb, 0.0)
                qblk.append(qb)

            # settle all constant loads so per-frame instructions don't each
            # carry per-DMA-lane waits (ACT has a small sync-wait budget)
            tc.strict_bb_all_engine_barrier()

            for b in [bb % BPC for bb in range(BPC * repeat)]:
                ktc = cache.tile([128, 2, S], f32, tag="ktc")
                vtc = cache.tile([128, 2, S], f32, tag="vtc")
                for s in range(S):
                    slab = slabs.tile([128, 2, HWD], f32, tag="slab")
                    gsum = small.tile([128, 2], f32, tag="gsum")
                    for j in range(2):
                        nc.sync.dma_start(
                            out=slab[:, j, :],
                            in_=ff[b, s, j * 128:(j + 1) * 128, :])
                    if probe == "dma":
                        for j in range(2):
                            nc.scalar.dma_start(
                                out=cross[b, s, j * 128:(j + 1) * 128, :],
                                in_=slab[:, j, :])
                        continue
                    # spatial-sum: half on ACT (in-place copy + accum), half on DVE
                    nc.scalar.activation(
                        out=slab[:, 0, :], in_=slab[:, 0, :],
                        func=AFT.Copy, accum_out=gsum[:, 0:1])
                    nc.vector.reduce_sum(
                        out=gsum[:, 1:2], in_=slab[:, 1, :], axis=AX.X)
                    # g = gsum/4096 + temporal_pos[s]
                    g = small.tile([128, 2], f32, tag="g")
                    for j in range(2):
                        nc.scalar.activation(
                            out=g[:, j:j + 1], in_=gsum[:, j:j + 1],
                            func=AFT.Identity, bias=tp_sb[:, s, j:j + 1],
                            scale=1.0 / HWD)
                    # q/k/v projections -> one PSUM tile, col = w*2 + etile
                    qkvp = psA.tile([128, 6], f32)
                    for w, nm in enumerate(("q", "k", "v")):
                        for t in range(2):
                            col = w * 2 + t
                            nc.tensor.matmul(
                                qkvp[:, col:col + 1],
                                lhsT=wsb[nm, 0][:, t * 128:(t + 1) * 128],
                                rhs=g[:, 0:1], start=True, stop=False)
                            nc.tensor.matmul(
                                qkvp[:, col:col + 1],
                                lhsT=wsb[nm, 1][:, t * 128:(t + 1) * 128],
                                rhs=g[:, 1:2], start=False, stop=True)
                    # q (+bq) scattered into block-diag; k/v (+bias) into caches
                    for j in range(2):
                        for r in range(4):
                            pslc = slice(r * 32, (r + 1) * 32)
                            nc.scalar.activation(
                                out=qblk[j][pslc, r:r + 1], in_=qkvp[pslc, j:j + 1],
                                func=AFT.Identity, bias=bc_sb[pslc, j:j + 1])
                        nc.scalar.activation(
                            out=ktc[:, j, s:s + 1], in_=qkvp[:, 2 + j:3 + j],
                            func=AFT.Identity, bias=bc_sb[:, 2 + j:3 + j])
                        nc.scalar.activation(
                            out=vtc[:, j, s:s + 1], in_=qkvp[:, 4 + j:5 + j],
                            func=AFT.Identity, bias=bc_sb[:, 4 + j:5 + j])
                    # scores[r, j, t] = q_h . k_t  (h = j*4+r), causal: t <= s
                    sc = psS.tile([4, 2, S], f32)
                    for j in range(2):
                        nc.tensor.matmul(
                            sc[:, j, 0:s + 1], lhsT=qblk[j],
                            rhs=ktc[:, j, 0:s + 1], start=True, stop=True)
                    # softmax over t (no max-subtraction: |score/sqrt(d)| < ~4)
                    pr = small.tile([4, 2, S], f32, tag="pr")
                    nc.scalar.activation(
                        out=pr[:, :, 0:s + 1], in_=sc[:, :, 0:s + 1],
                        func=AFT.Exp, scale=SCALE)
                    ssum = small.tile([4, 2], f32, tag="ssum")
                    nc.vector.reduce_sum(out=ssum, in_=pr[:, :, 0:s + 1], axis=AX.X)
                    rcp = small.tile([4, 2], f32, tag="rcp")
                    nc.vector.reciprocal(out=rcp, in_=ssum)
                    for j in range(2):
                        base = ((b * S + s) * 2 + j) * S
                        nc.vector.tensor_scalar_mul(
                            out=attn_acc[:, base:base + s + 1],
                            in0=pr[:, j, 0:s + 1], scalar1=rcp[:, j:j + 1])
                    # expand probs to partition space, weight v cache, reduce
                    pe = psE.tile([128, 2, S], f32)
                    for j in range(2):
                        base = ((b * S + s) * 2 + j) * S
                        nc.tensor.matmul(
                            pe[:, j, 0:s + 1], lhsT=er_sb,
                            rhs=attn_acc[:, base:base + s + 1],
                            start=True, stop=True)
                    tmp = small.tile([128, 2, S], f32, tag="tmp")
                    nc.vector.tensor_mul(
                        out=tmp[:, :, 0:s + 1], in0=vtc[:, :, 0:s + 1],
                        in1=pe[:, :, 0:s + 1])
                    att = small.tile([128, 2], f32, tag="att")
                    nc.vector.reduce_sum(out=att, in_=tmp[:, :, 0:s + 1], axis=AX.X)
                    # output projection + bo
                    ogp = psO.tile([128, 2], f32)
                    for t in range(2):
                        nc.tensor.matmul(
                            ogp[:, t:t + 1], lhsT=wsb["o", 0][:, t * 128:(t + 1) * 128],
                            rhs=att[:, 0:1], start=True, stop=False)
                        nc.tensor.matmul(
                            ogp[:, t:t + 1], lhsT=wsb["o", 1][:, t * 128:(t + 1) * 128],
                            rhs=att[:, 1:2], start=False, stop=True)
                    ogs = small.tile([128, 2], f32, tag="ogs")
                    for j in range(2):
                        nc.scalar.activation(
                            out=ogs[:, j:j + 1], in_=ogp[:, j:j + 1],
                            func=AFT.Identity, bias=bc_sb[:, 6 + j:7 + j])
                    # broadcast-add onto the resident slab (DVE half, GPSIMD
                    # half — ACT is busy with reduces/smalls), then store
                    nc.vector.tensor_scalar_add(
                        out=slab[:, 0, :], in0=slab[:, 0, :],
                        scalar1=ogs[:, 0:1])
                    nc.gpsimd.tensor_scalar_add(
                        out=slab[:, 1, :], in0=slab[:, 1, :],
                        scalar1=ogs[:, 1:2])
                    for j in range(2):
                        nc.scalar.dma_start(
                            out=cross[b, s, j * 128:(j + 1) * 128, :],
                            in_=slab[:, j, :])

            # attention output: [4,(b,s,j,t)] -> attn[b, j*4+r, s, t]
            attn_view = attn_acc[:].rearrange(
                "r (b s j t) -> r b s j t", b=BPC, s=S, j=2)
            for b in range(BPC):
                for j in range(2):
                    nc.sync.dma_start(
                        out=attn[b, j * 4:(j + 1) * 4, :, :],
                        in_=attn_view[:, b, :, j, :])

    nc.finalize()
    return nc


def _get_nc(repeat=1, probe=None):
    key = ("nc", repeat, probe)
    if key not in _CACHE:
        _CACHE[key] = _build_nc(repeat, probe)
    return _CACHE[key]


def _prep_in_maps(inputs):
    ff = np.ascontiguousarray(
        np.asarray(inputs["frame_features"], dtype=np.float32)).reshape(B, S, E, HWD)
    blocks = []
    for key in ("Wq", "Wk", "Wv", "Wo"):
        wt = np.asarray(inputs[key], dtype=np.float32).T  # [f, e]
        blocks.append(wt[0:128, :])
        blocks.append(wt[128:256, :])
    tp = np.asarray(inputs["temporal_pos"], dtype=np.float32)
    blocks.append(tp.reshape(S, 2, 128).transpose(2, 0, 1).reshape(128, 12))
    bs = [np.asarray(inputs[k], dtype=np.float32).reshape(2, 128)
          for k in ("bq", "bk", "bv", "bo")]
    blocks.append(np.stack(bs, axis=0).transpose(2, 0, 1).reshape(128, 8))
    er = np.zeros((128, 128), dtype=np.float32)
    er[0:4] = np.repeat(np.eye(4, dtype=np.float32), 32, axis=1)
    blocks.append(er)
    consts = np.ascontiguousarray(np.concatenate(blocks, axis=1))
    assert consts.shape == (128, 2196), consts.shape

    in_maps = []
    for c in range(NCORES):
        m = {"consts": consts}
        m["ff"] = np.ascontiguousarray(ff[c * BPC:(c + 1) * BPC])
        in_maps.append(m)
    return in_maps


# ---------------------------------------------------------------------------
# Workaround: this container's walrus build accepts at most ONE sync-wait per
# instruction ("Too many sync wait commands" in setupSyncWait otherwise), but
# Tile's semaphore assignment attaches several.  Split extras onto single-wait
# NoOps inserted just before the instruction (same engine -> order preserved).
# ---------------------------------------------------------------------------

def _split_multiwaits(bir_json: bytes) -> bytes:
    import orjson

    js = orjson.loads(bir_json)
    for fn in js.get("functions", []):
        for blk in fn.get("blocks", []):
            out = []
            for inst in blk.get("instructions", []):
                si = inst.get("sync_info")
                waits = (si or {}).get("on_wait") or []
                if len(waits) > 1:
                    for k, w in enumerate(waits[:-1]):
                        out.append({
                            "engine": inst["engine"],
                            "ins": [],
                            "name": f"{inst['name']}-sw{k}",
                            "opcode": "NoOp",
                            "outs": [],
                            "sync_info": {"on_update": [], "on_wait": [w]},
                        })
                    si["on_wait"] = [waits[-1]]
                out.append(inst)
            blk["instructions"] = out
    return orjson.dumps(js)


def _install_wait_splitter():
    import concourse.bass_utils as bu
    import concourse.bass2jax as b2j

    if getattr(bu.compile_bir_kernel, "_split_wrapped", False):
        return
    orig = bu.compile_bir_kernel

    def wrapped(bir_json, tmpdir, neff_name="file.neff"):
        return orig(_split_multiwaits(bir_json), tmpdir, neff_name)

    wrapped._split_wrapped = True
    bu.compile_bir_kernel = wrapped
    b2j.compile_bir_kernel = wrapped


def _run(inputs):
    from concourse.bass_utils import run_bass_kernel_spmd

    _install_wait_splitter()

    # The axon NTFF-profile hook is not present in this container; make sure
    # run_bass_kernel_spmd never takes the trace path.
    os.environ["BASS_NEVER_TRACE"] = "1"
    nc = _get_nc()
    in_maps = _prep_in_maps(inputs)
    res = run_bass_kernel_spmd(
        nc, in_maps, core_ids=list(range(NCORES)), trace=False)
    cross = np.concatenate(
        [r["cross"] for r in res.results], axis=0).reshape(B, S, E, 64, 64)
    attn = np.concatenate([r["attn"] for r in res.results], axis=0)
    return cross, attn, res


def kernel(**inputs):
    cross, attn, _ = _run(inputs)
    return cross, attn


# ---------------------------------------------------------------------------
# Benchmark path (test.py only): cached jit, device-resident inputs, no
# donation (every output element is written by the kernel), so repeated
# executions measure pure device time.
# ---------------------------------------------------------------------------

def _get_exec(repeat=1, probe=None):
    key = ("exec", repeat, probe)
    if key in _CACHE:
        return _CACHE[key]
    import jax
    from concourse import bass2jax as b2j
    from concourse import mybir

    _install_wait_splitter()
    b2j.install_neuronx_cc_hook()
    nc = _get_nc(repeat, probe)
    partition_name = nc.partition_id_tensor.name if nc.partition_id_tensor else None

    in_names, out_names, out_avals = [], [], []
    for alloc in nc.m.functions[0].allocations:
        if not isinstance(alloc, mybir.MemoryLocationSet):
            continue
        name = alloc.memorylocations[0].name
        if alloc.kind == "ExternalInput":
            if name != partition_name:
                in_names.append(name)
        elif alloc.kind == "ExternalOutput":
            out_names.append(name)
            out_avals.append(jax.core.ShapedArray(
                tuple(alloc.tensor_shape), mybir.dt.np(alloc.dtype)))
    n_params = len(in_names)
    all_in = in_names + out_names
    if partition_name is not None:
        all_in = all_in + [partition_name]

    def _body(*args):
        operands = list(args)
        if partition_name is not None:
            operands.append(b2j.partition_id_tensor())
        return tuple(b2j._bass_exec_p.bind(
            *operands,
            out_avals=tuple(out_avals),
            in_names=tuple(all_in),
            out_names=tuple(out_names),
            lowering_input_output_aliases=(),
            sim_require_finite=True,
            sim_require_nnan=True,
            nc=nc,
        ))

    devices = jax.devices()[:NCORES]
    mesh = b2j.Mesh(np.asarray(devices), ("core",))
    P = b2j.PartitionSpec
    in_specs = (P("core"),) * (n_params + len(out_names))
    out_specs = (P("core"),) * len(out_names)
    fn = jax.jit(
        b2j.shard_map(_body, mesh=mesh, in_specs=in_specs,
                      out_specs=out_specs, check_rep=False),
        keep_unused=True)
    _CACHE[key] = (fn, mesh, in_names, out_names, out_avals, n_params)
    return _CACHE[key]


def _dev_args(inputs, repeat=1, probe=None):
    import jax
    import jax.numpy as jnp
    from jax.sharding import NamedSharding
    from jax.sharding import PartitionSpec as P

    fn, mesh, in_names, out_names, out_avals, n_params = _get_exec(repeat, probe)
    in_maps = _prep_in_maps(inputs)
    sh = NamedSharding(mesh, P("core"))
    dev_in = []
    for name in in_names:
        cat = np.concatenate(
            [np.asarray(in_maps[c][name]) for c in range(NCORES)], axis=0)
        dev_in.append(jax.device_put(cat, sh))
    zeros = jax.jit(
        lambda: tuple(
            jnp.zeros((NCORES * a.shape[0], *a.shape[1:]), a.dtype)
            for a in out_avals),
        out_shardings=tuple(sh for _ in out_avals))()
    return fn, out_names, (*dev_in, *zeros)


def _time_pipelined(fn, args, iters):
    import time as _time

    import jax

    jax.block_until_ready(fn(*args))
    best = float("inf")
    for _ in range(3):
        t0 = _time.perf_counter()
        last = None
        for _ in range(iters):
            last = fn(*args)
        jax.block_until_ready(last)
        best = min(best, (_time.perf_counter() - t0) / iters)
    return best


def bench_repeat(inputs, r_lo=8, r_hi=32, iters=12, probe=None):
    """Per-pass device time from the slope between R-repeat NEFF variants.

    The R-repeat NEFF runs the full computation R times back-to-back inside
    one device execution, so (T(r_hi) - T(r_lo)) / (r_hi - r_lo) cancels all
    dispatch/RPC overhead.
    """
    f_lo, out_names, args_lo = _dev_args(inputs, repeat=r_lo, probe=probe)
    f_hi, _, args_hi = _dev_args(inputs, repeat=r_hi, probe=probe)
    t_lo = _time_pipelined(f_lo, args_lo, iters)
    t_hi = _time_pipelined(f_hi, args_hi, iters)
    per_pass_ns = (t_hi - t_lo) * 1e9 / (r_hi - r_lo)

    import jax
    outs = f_lo(*args_lo)
    jax.block_until_ready(outs)
    cross = np.asarray(outs[out_names.index("cross")]).reshape(B, S, E, 64, 64)
    attn = np.asarray(outs[out_names.index("attn")]).reshape(B, NH, S, S)
    return cross, attn, per_pass_ns, (t_lo, t_hi)


def bench(inputs, iters=20, warmup=3):
    """Returns (cross, attn, per_iter_ns_pipelined, per_call_ns_list)."""
    import time as _time

    import jax
    import jax.numpy as jnp
    from jax.sharding import NamedSharding
    from jax.sharding import PartitionSpec as P

    fn, mesh, in_names, out_names, out_avals, n_params = _get_exec()
    in_maps = _prep_in_maps(inputs)
    sh = NamedSharding(mesh, P("core"))

    dev_in = []
    for i, name in enumerate(in_names):
        cat = np.concatenate(
            [np.asarray(in_maps[c][name]) for c in range(NCORES)], axis=0)
        dev_in.append(jax.device_put(cat, sh))
    zeros = jax.jit(
        lambda: tuple(
            jnp.zeros((NCORES * a.shape[0], *a.shape[1:]), a.dtype)
            for a in out_avals),
        out_shardings=tuple(sh for _ in out_avals))()
    args = (*dev_in, *zeros)

    outs = fn(*args)
    jax.block_until_ready(outs)

    per_call = []
    for _ in range(warmup):
        jax.block_until_ready(fn(*args))
    for _ in range(5):
        t0 = _time.perf_counter()
        jax.block_until_ready(fn(*args))
        per_call.append((_time.perf_counter() - t0) * 1e9)

    t0 = _time.perf_counter()
    last = None
    for _ in range(iters):
        last = fn(*args)
    jax.block_until_ready(last)
    pipelined_ns = (_time.perf_counter() - t0) * 1e9 / iters

    cross = np.asarray(outs[out_names.index("cross")]).reshape(
        B, S, E, 64, 64)
    attn = np.asarray(outs[out_names.index("attn")]).reshape(B, NH, S, S)
    return cross, attn, pipelined_ns, per_call


# revision 27
# speedup vs baseline: 2.5098x; 2.5098x over previous
"""CrossFrameAttention Trainium2 kernel.

Reference computation (B=16, S=6, E=256, heads=8, spatial 64x64):
  g    = mean_{h,w}(frame_features) + temporal_pos          [B,S,E]
  q/k/v= g @ W{q,k,v}.T + b                                 [B,S,E]
  attn = causal softmax(q k^T / sqrt(32)) per head          [B,8,S,S]
  out_g= (attn @ v) @ Wo.T + bo                             [B,S,E]
  cross= frame_features + out_g[..., None, None]
  returns (cross, attn)

Strategy: pure batch data-parallelism, 2 batches per core on 8 cores.
Single-pass streaming per (batch, frame): DMA the 4 MB frame slab into
SBUF, reduce it for the spatial mean, run the (tiny) incremental causal
attention row for that frame on-chip, broadcast-add out_g onto the slab
still resident in SBUF, DMA it out.  The 402 MB input is read exactly
once and written exactly once -> HBM roofline ~100 MB/core.

Channel layout on chip: e = j*128 + p (j in {0,1} selects the half,
p is the SBUF partition).  Head h = j*4 + p//32.

kernel() takes FULL inputs and returns the FULL (cross, attn) outputs.
"""

import math
import os

import numpy as np

B, S, E, HWD = 16, 6, 256, 4096
NH, HD = 8, 32
NCORES = 8
BPC = B // NCORES  # batches per core

_CACHE = {}


def _build_nc(repeat=1, probe=None):
    import concourse.bass as bass
    import concourse.tile as tile
    from concourse import mybir

    f32 = mybir.dt.float32
    AFT = mybir.ActivationFunctionType
    AX = mybir.AxisListType

    nc = bass.Bass(trn_type="TRN2")

    ff = nc.dram_tensor("ff", [BPC, S, E, HWD], f32, kind="ExternalInput")
    # all small constants packed into one tensor -> one DMA -> one sem lane:
    # cols 0:2048   = W{q,k,v,o}.T halves, block (w*2+j)*256 : +256
    # cols 2048:2060 = temporal_pos cols (s*2+j)
    # cols 2060:2068 = biases cols (w*2+j)
    # cols 2068:2196 = head-expand matrix er on partitions 0:4
    CW, CTP, CBC, CER = 0, 2048, 2060, 2068
    CTOT = 2196
    consts = nc.dram_tensor("consts", [128, CTOT], f32, kind="ExternalInput")
    cross = nc.dram_tensor("cross", [BPC, S, E, HWD], f32, kind="ExternalOutput")
    attn = nc.dram_tensor("attn", [BPC, NH, S, S], f32, kind="ExternalOutput")

    SCALE = 1.0 / math.sqrt(HD)

    with tile.TileContext(nc) as tc:
        with tc.tile_pool(name="const", bufs=1) as const, \
             tc.tile_pool(name="slabs", bufs=5) as slabs, \
             tc.tile_pool(name="small", bufs=6) as small, \
             tc.tile_pool(name="cache", bufs=2) as cache, \
             tc.tile_pool(name="psA", bufs=4, space="PSUM") as psA, \
             tc.tile_pool(name="psS", bufs=4, space="PSUM") as psS:

            # --- constants: one DMA ---
            cs = const.tile([128, CTOT], f32, tag="consts")
            nc.sync.dma_start(out=cs, in_=consts[:])
            wsb = {}
            for w, nm in enumerate(("q", "k", "v", "o")):
                for j in range(2):
                    base = CW + (w * 2 + j) * 256
                    wsb[nm, j] = cs[:, base:base + 256]
            tp_sb = cs[:, CTP:CTP + 12].rearrange("p (s j) -> p s j", s=S)
            bc_sb = cs[:, CBC:CBC + 8]
            er_sb = cs[0:4, CER:CER + 128]

            # attention probabilities accumulator [r, (b, s, j, t)]
            attn_acc = const.tile([4, BPC * S * 2 * S], f32, tag="attn_acc")
            nc.vector.memset(attn_acc, 0.0)
            # block-diagonal q (per j half): [p, r] nonzero only when p//32==r
            qblk = []
            for j in range(2):
                qb = const.tile([128, 4], f32, tag=f"qblk{j}")
                nc.vector.memset(qb, 0.0)
                qblk.append(qb)

            # settle all constant loads so per-frame instructions don't each
            # carry per-DMA-lane waits (ACT has a small sync-wait budget)
            tc.strict_bb_all_engine_barrier()

            for b in [bb % BPC for bb in range(BPC * repeat)]:
                ktc = cache.tile([128, 2, S], f32, tag="ktc")
                vtc = cache.tile([128, 2, S], f32, tag="vtc")
                for s in range(S):
                    slab = slabs.tile([128, 2, HWD], f32, tag="slab")
                    gsum = small.tile([128, 2], f32, tag="gsum")
                    for j in range(2):
                        nc.sync.dma_start(
                            out=slab[:, j, :],
                            in_=ff[b, s, j * 128:(j + 1) * 128, :])
                    if probe == "dma":
                        for j in range(2):
                            nc.scalar.dma_start(
                                out=cross[b, s, j * 128:(j + 1) * 128, :],
                                in_=slab[:, j, :])
                        continue
                    # spatial-sum: half on ACT (in-place copy + accum), half on DVE
                    nc.scalar.activation(
                        out=slab[:, 0, :], in_=slab[:, 0, :],
                        func=AFT.Copy, accum_out=gsum[:, 0:1])
                    nc.vector.reduce_sum(
                        out=gsum[:, 1:2], in_=slab[:, 1, :], axis=AX.X)
                    # g = gsum/4096 + temporal_pos[s]
                    g = small.tile([128, 2], f32, tag="g")
                    for j in range(2):
                        nc.scalar.activation(
                            out=g[:, j:j + 1], in_=gsum[:, j:j + 1],
                            func=AFT.Identity, bias=tp_sb[:, s, j:j + 1],
                            scale=1.0 / HWD)
                    # q/k/v projections -> one PSUM tile, col = w*2 + etile
                    # (cols 6:8 used later for the output projection)
                    qkvp = psA.tile([128, 8], f32)
                    for w, nm in enumerate(("q", "k", "v")):
                        for t in range(2):
                            col = w * 2 + t
                            nc.tensor.matmul(
                                qkvp[:, col:col + 1],
                                lhsT=wsb[nm, 0][:, t * 128:(t + 1) * 128],
                                rhs=g[:, 0:1], start=True, stop=False)
                            nc.tensor.matmul(
                                qkvp[:, col:col + 1],
                                lhsT=wsb[nm, 1][:, t * 128:(t + 1) * 128],
                                rhs=g[:, 1:2], start=False, stop=True)
                    # q (+bq) scattered into block-diag; k/v (+bias) into caches
                    for j in range(2):
                        for r in range(4):
                            pslc = slice(r * 32, (r + 1) * 32)
                            nc.scalar.activation(
                                out=qblk[j][pslc, r:r + 1], in_=qkvp[pslc, j:j + 1],
                                func=AFT.Identity, bias=bc_sb[pslc, j:j + 1])
                        nc.scalar.activation(
                            out=ktc[:, j, s:s + 1], in_=qkvp[:, 2 + j:3 + j],
                            func=AFT.Identity, bias=bc_sb[:, 2 + j:3 + j])
                        nc.scalar.activation(
                            out=vtc[:, j, s:s + 1], in_=qkvp[:, 4 + j:5 + j],
                            func=AFT.Identity, bias=bc_sb[:, 4 + j:5 + j])
                    # scores[r, j, t] = q_h . k_t  (h = j*4+r), causal: t <= s
                    # one PSUM tile: pe in cols 0:6, scores in cols 6:12
                    spe = psS.tile([128, 2, 2 * S], f32)
                    sc = spe[0:4, :, S:2 * S]
                    pe = spe[:, :, 0:S]
                    for j in range(2):
                        nc.tensor.matmul(
                            sc[:, j, 0:s + 1], lhsT=qblk[j],
                            rhs=ktc[:, j, 0:s + 1], start=True, stop=True)
                    # softmax over t (no max-subtraction: |score/sqrt(d)| < ~4);
                    # Exp's accum_out yields the per-head sum for free
                    pr = small.tile([4, 2, S], f32, tag="pr")
                    ssum = small.tile([4, 2], f32, tag="ssum")
                    for j in range(2):
                        nc.scalar.activation(
                            out=pr[:, j, 0:s + 1], in_=sc[:, j, 0:s + 1],
                            func=AFT.Exp, scale=SCALE,
                            accum_out=ssum[:, j:j + 1])
                    rcp = small.tile([4, 2], f32, tag="rcp")
                    nc.vector.reciprocal(out=rcp, in_=ssum)
                    for j in range(2):
                        base = ((b * S + s) * 2 + j) * S
                        nc.vector.tensor_scalar_mul(
                            out=attn_acc[:, base:base + s + 1],
                            in0=pr[:, j, 0:s + 1], scalar1=rcp[:, j:j + 1])
                    # expand probs to partition space, weight v cache, reduce
                    for j in range(2):
                        base = ((b * S + s) * 2 + j) * S
                        nc.tensor.matmul(
                            pe[:, j, 0:s + 1], lhsT=er_sb,
                            rhs=attn_acc[:, base:base + s + 1],
                            start=True, stop=True)
                    tmp = small.tile([128, 2, S], f32, tag="tmp")
                    nc.vector.tensor_mul(
                        out=tmp[:, :, 0:s + 1], in0=vtc[:, :, 0:s + 1],
                        in1=pe[:, :, 0:s + 1])
                    att = small.tile([128, 2], f32, tag="att")
                    nc.vector.reduce_sum(out=att, in_=tmp[:, :, 0:s + 1], axis=AX.X)
                    # output projection + bo (cols 6:8 of the qkv PSUM tile)
                    ogp = qkvp[:, 6:8]
                    for t in range(2):
                        nc.tensor.matmul(
                            ogp[:, t:t + 1], lhsT=wsb["o", 0][:, t * 128:(t + 1) * 128],
                            rhs=att[:, 0:1], start=True, stop=False)
                        nc.tensor.matmul(
                            ogp[:, t:t + 1], lhsT=wsb["o", 1][:, t * 128:(t + 1) * 128],
                            rhs=att[:, 1:2], start=False, stop=True)
                    ogs = small.tile([128, 2], f32, tag="ogs")
                    for j in range(2):
                        nc.scalar.activation(
                            out=ogs[:, j:j + 1], in_=ogp[:, j:j + 1],
                            func=AFT.Identity, bias=bc_sb[:, 6 + j:7 + j])
                    # broadcast-add onto the resident slab (DVE 2x-mode half,
                    # ACT half), then store
                    nc.vector.tensor_scalar_add(
                        out=slab[:, 0, :], in0=slab[:, 0, :],
                        scalar1=ogs[:, 0:1])
                    nc.scalar.activation(
                        out=slab[:, 1, :], in_=slab[:, 1, :],
                        func=AFT.Identity, bias=ogs[:, 1:2])
                    for j in range(2):
                        nc.scalar.dma_start(
                            out=cross[b, s, j * 128:(j + 1) * 128, :],
                            in_=slab[:, j, :])

            # attention output: [4,(b,s,j,t)] -> attn[b, j*4+r, s, t]
            attn_view = attn_acc[:].rearrange(
                "r (b s j t) -> r b s j t", b=BPC, s=S, j=2)
            for b in range(BPC):
                for j in range(2):
                    nc.sync.dma_start(
                        out=attn[b, j * 4:(j + 1) * 4, :, :],
                        in_=attn_view[:, b, :, j, :])

    nc.finalize()
    return nc


def _get_nc(repeat=1, probe=None):
    key = ("nc", repeat, probe)
    if key not in _CACHE:
        _CACHE[key] = _build_nc(repeat, probe)
    return _CACHE[key]


def _prep_in_maps(inputs):
    ff = np.ascontiguousarray(
        np.asarray(inputs["frame_features"], dtype=np.float32)).reshape(B, S, E, HWD)
    blocks = []
    for key in ("Wq", "Wk", "Wv", "Wo"):
        wt = np.asarray(inputs[key], dtype=np.float32).T  # [f, e]
        blocks.append(wt[0:128, :])
        blocks.append(wt[128:256, :])
    tp = np.asarray(inputs["temporal_pos"], dtype=np.float32)
    blocks.append(tp.reshape(S, 2, 128).transpose(2, 0, 1).reshape(128, 12))
    bs = [np.asarray(inputs[k], dtype=np.float32).reshape(2, 128)
          for k in ("bq", "bk", "bv", "bo")]
    blocks.append(np.stack(bs, axis=0).transpose(2, 0, 1).reshape(128, 8))
    er = np.zeros((128, 128), dtype=np.float32)
    er[0:4] = np.repeat(np.eye(4, dtype=np.float32), 32, axis=1)
    blocks.append(er)
    consts = np.ascontiguousarray(np.concatenate(blocks, axis=1))
    assert consts.shape == (128, 2196), consts.shape

    in_maps = []
    for c in range(NCORES):
        m = {"consts": consts}
        m["ff"] = np.ascontiguousarray(ff[c * BPC:(c + 1) * BPC])
        in_maps.append(m)
    return in_maps


# ---------------------------------------------------------------------------
# Workaround: this container's walrus build accepts at most ONE sync-wait per
# instruction ("Too many sync wait commands" in setupSyncWait otherwise), but
# Tile's semaphore assignment attaches several.  Split extras onto single-wait
# NoOps inserted just before the instruction (same engine -> order preserved).
# ---------------------------------------------------------------------------

def _split_multiwaits(bir_json: bytes) -> bytes:
    import orjson

    js = orjson.loads(bir_json)
    for fn in js.get("functions", []):
        for blk in fn.get("blocks", []):
            out = []
            for inst in blk.get("instructions", []):
                si = inst.get("sync_info")
                waits = (si or {}).get("on_wait") or []
                if len(waits) > 1:
                    for k, w in enumerate(waits[:-1]):
                        out.append({
                            "engine": inst["engine"],
                            "ins": [],
                            "name": f"{inst['name']}-sw{k}",
                            "opcode": "NoOp",
                            "outs": [],
                            "sync_info": {"on_update": [], "on_wait": [w]},
                        })
                    si["on_wait"] = [waits[-1]]
                out.append(inst)
            blk["instructions"] = out
    return orjson.dumps(js)


def _install_wait_splitter():
    import concourse.bass_utils as bu
    import concourse.bass2jax as b2j

    if getattr(bu.compile_bir_kernel, "_split_wrapped", False):
        return
    orig = bu.compile_bir_kernel

    def wrapped(bir_json, tmpdir, neff_name="file.neff"):
        return orig(_split_multiwaits(bir_json), tmpdir, neff_name)

    wrapped._split_wrapped = True
    bu.compile_bir_kernel = wrapped
    b2j.compile_bir_kernel = wrapped


def _run(inputs):
    from concourse.bass_utils import run_bass_kernel_spmd

    _install_wait_splitter()

    # The axon NTFF-profile hook is not present in this container; make sure
    # run_bass_kernel_spmd never takes the trace path.
    os.environ["BASS_NEVER_TRACE"] = "1"
    nc = _get_nc()
    in_maps = _prep_in_maps(inputs)
    res = run_bass_kernel_spmd(
        nc, in_maps, core_ids=list(range(NCORES)), trace=False)
    cross = np.concatenate(
        [r["cross"] for r in res.results], axis=0).reshape(B, S, E, 64, 64)
    attn = np.concatenate([r["attn"] for r in res.results], axis=0)
    return cross, attn, res


def kernel(**inputs):
    cross, attn, _ = _run(inputs)
    return cross, attn


# ---------------------------------------------------------------------------
# Benchmark path (test.py only): cached jit, device-resident inputs, no
# donation (every output element is written by the kernel), so repeated
# executions measure pure device time.
# ---------------------------------------------------------------------------

def _get_exec(repeat=1, probe=None):
    key = ("exec", repeat, probe)
    if key in _CACHE:
        return _CACHE[key]
    import jax
    from concourse import bass2jax as b2j
    from concourse import mybir

    _install_wait_splitter()
    b2j.install_neuronx_cc_hook()
    nc = _get_nc(repeat, probe)
    partition_name = nc.partition_id_tensor.name if nc.partition_id_tensor else None

    in_names, out_names, out_avals = [], [], []
    for alloc in nc.m.functions[0].allocations:
        if not isinstance(alloc, mybir.MemoryLocationSet):
            continue
        name = alloc.memorylocations[0].name
        if alloc.kind == "ExternalInput":
            if name != partition_name:
                in_names.append(name)
        elif alloc.kind == "ExternalOutput":
            out_names.append(name)
            out_avals.append(jax.core.ShapedArray(
                tuple(alloc.tensor_shape), mybir.dt.np(alloc.dtype)))
    n_params = len(in_names)
    all_in = in_names + out_names
    if partition_name is not None:
        all_in = all_in + [partition_name]

    def _body(*args):
        operands = list(args)
        if partition_name is not None:
            operands.append(b2j.partition_id_tensor())
        return tuple(b2j._bass_exec_p.bind(
            *operands,
            out_avals=tuple(out_avals),
            in_names=tuple(all_in),
            out_names=tuple(out_names),
            lowering_input_output_aliases=(),
            sim_require_finite=True,
            sim_require_nnan=True,
            nc=nc,
        ))

    devices = jax.devices()[:NCORES]
    mesh = b2j.Mesh(np.asarray(devices), ("core",))
    P = b2j.PartitionSpec
    in_specs = (P("core"),) * (n_params + len(out_names))
    out_specs = (P("core"),) * len(out_names)
    fn = jax.jit(
        b2j.shard_map(_body, mesh=mesh, in_specs=in_specs,
                      out_specs=out_specs, check_rep=False),
        keep_unused=True)
    _CACHE[key] = (fn, mesh, in_names, out_names, out_avals, n_params)
    return _CACHE[key]


def _dev_args(inputs, repeat=1, probe=None):
    import jax
    import jax.numpy as jnp
    from jax.sharding import NamedSharding
    from jax.sharding import PartitionSpec as P

    fn, mesh, in_names, out_names, out_avals, n_params = _get_exec(repeat, probe)
    in_maps = _prep_in_maps(inputs)
    sh = NamedSharding(mesh, P("core"))
    dev_in = []
    for name in in_names:
        cat = np.concatenate(
            [np.asarray(in_maps[c][name]) for c in range(NCORES)], axis=0)
        dev_in.append(jax.device_put(cat, sh))
    zeros = jax.jit(
        lambda: tuple(
            jnp.zeros((NCORES * a.shape[0], *a.shape[1:]), a.dtype)
            for a in out_avals),
        out_shardings=tuple(sh for _ in out_avals))()
    return fn, out_names, (*dev_in, *zeros)


def _time_pipelined(fn, args, iters):
    import time as _time

    import jax

    jax.block_until_ready(fn(*args))
    best = float("inf")
    for _ in range(3):
        t0 = _time.perf_counter()
        last = None
        for _ in range(iters):
            last = fn(*args)
        jax.block_until_ready(last)
        best = min(best, (_time.perf_counter() - t0) / iters)
    return best


def bench_repeat(inputs, r_lo=8, r_hi=32, iters=12, probe=None):
    """Per-pass device time from the slope between R-repeat NEFF variants.

    The R-repeat NEFF runs the full computation R times back-to-back inside
    one device execution, so (T(r_hi) - T(r_lo)) / (r_hi - r_lo) cancels all
    dispatch/RPC overhead.
    """
    f_lo, out_names, args_lo = _dev_args(inputs, repeat=r_lo, probe=probe)
    f_hi, _, args_hi = _dev_args(inputs, repeat=r_hi, probe=probe)
    t_lo = _time_pipelined(f_lo, args_lo, iters)
    t_hi = _time_pipelined(f_hi, args_hi, iters)
    per_pass_ns = (t_hi - t_lo) * 1e9 / (r_hi - r_lo)

    import jax
    outs = f_lo(*args_lo)
    jax.block_until_ready(outs)
    cross = np.asarray(outs[out_names.index("cross")]).reshape(B, S, E, 64, 64)
    attn = np.asarray(outs[out_names.index("attn")]).reshape(B, NH, S, S)
    return cross, attn, per_pass_ns, (t_lo, t_hi)


def bench(inputs, iters=20, warmup=3):
    """Returns (cross, attn, per_iter_ns_pipelined, per_call_ns_list)."""
    import time as _time

    import jax
    import jax.numpy as jnp
    from jax.sharding import NamedSharding
    from jax.sharding import PartitionSpec as P

    fn, mesh, in_names, out_names, out_avals, n_params = _get_exec()
    in_maps = _prep_in_maps(inputs)
    sh = NamedSharding(mesh, P("core"))

    dev_in = []
    for i, name in enumerate(in_names):
        cat = np.concatenate(
            [np.asarray(in_maps[c][name]) for c in range(NCORES)], axis=0)
        dev_in.append(jax.device_put(cat, sh))
    zeros = jax.jit(
        lambda: tuple(
            jnp.zeros((NCORES * a.shape[0], *a.shape[1:]), a.dtype)
            for a in out_avals),
        out_shardings=tuple(sh for _ in out_avals))()
    args = (*dev_in, *zeros)

    outs = fn(*args)
    jax.block_until_ready(outs)

    per_call = []
    for _ in range(warmup):
        jax.block_until_ready(fn(*args))
    for _ in range(5):
        t0 = _time.perf_counter()
        jax.block_until_ready(fn(*args))
        per_call.append((_time.perf_counter() - t0) * 1e9)

    t0 = _time.perf_counter()
    last = None
    for _ in range(iters):
        last = fn(*args)
    jax.block_until_ready(last)
    pipelined_ns = (_time.perf_counter() - t0) * 1e9 / iters

    cross = np.asarray(outs[out_names.index("cross")]).reshape(
        B, S, E, 64, 64)
    attn = np.asarray(outs[out_names.index("attn")]).reshape(B, NH, S, S)
    return cross, attn, pipelined_ns, per_call


# revision 32
# speedup vs baseline: 3.4353x; 1.3687x over previous
"""CrossFrameAttention Trainium2 kernel.

Reference computation (B=16, S=6, E=256, heads=8, spatial 64x64):
  g    = mean_{h,w}(frame_features) + temporal_pos          [B,S,E]
  q/k/v= g @ W{q,k,v}.T + b                                 [B,S,E]
  attn = causal softmax(q k^T / sqrt(32)) per head          [B,8,S,S]
  out_g= (attn @ v) @ Wo.T + bo                             [B,S,E]
  cross= frame_features + out_g[..., None, None]
  returns (cross, attn)

Strategy: pure batch data-parallelism, 2 batches per core on 8 cores.
Single-pass streaming per (batch, frame): DMA the 4 MB frame slab into
SBUF, reduce it for the spatial mean, run the (tiny) incremental causal
attention row for that frame on-chip, broadcast-add out_g onto the slab
still resident in SBUF, DMA it out.  The 402 MB input is read exactly
once and written exactly once -> HBM roofline ~100 MB/core.

Channel layout on chip: e = j*128 + p (j in {0,1} selects the half,
p is the SBUF partition).  Head h = j*4 + p//32.

kernel() takes FULL inputs and returns the FULL (cross, attn) outputs.
"""

import math
import os

import numpy as np

B, S, E, HWD = 16, 6, 256, 4096
NH, HD = 8, 32
NCORES = 8
BPC = B // NCORES  # batches per core

_CACHE = {}


def _build_nc(repeat=1, probe=None):
    import concourse.bass as bass
    import concourse.tile as tile
    from concourse import mybir

    f32 = mybir.dt.float32
    AFT = mybir.ActivationFunctionType
    AX = mybir.AxisListType

    nc = bass.Bass(trn_type="TRN2")

    ff = nc.dram_tensor("ff", [BPC, S, E, HWD], f32, kind="ExternalInput")
    # all small constants packed into one tensor -> one DMA -> one sem lane:
    # cols 0:2048   = W{q,k,v,o}.T halves, block (w*2+j)*256 : +256
    # cols 2048:2060 = temporal_pos cols (s*2+j)
    # cols 2060:2068 = biases cols (w*2+j)
    # cols 2068:2196 = head-expand matrix er on partitions 0:4
    CW, CTP, CBC, CER = 0, 2048, 2060, 2068
    CTOT = 2196
    consts = nc.dram_tensor("consts", [128, CTOT], f32, kind="ExternalInput")
    cross = nc.dram_tensor("cross", [BPC, S, E, HWD], f32, kind="ExternalOutput")
    attn = nc.dram_tensor("attn", [BPC, NH, S, S], f32, kind="ExternalOutput")

    SCALE = 1.0 / math.sqrt(HD)

    with tile.TileContext(nc) as tc:
        with tc.tile_pool(name="const", bufs=1) as const, \
             tc.tile_pool(name="slabs", bufs=5) as slabs, \
             tc.tile_pool(name="small", bufs=6) as small, \
             tc.tile_pool(name="cache", bufs=2) as cache, \
             tc.tile_pool(name="psA", bufs=2, space="PSUM") as psA, \
             tc.tile_pool(name="psS", bufs=3, space="PSUM") as psS, \
             tc.tile_pool(name="psO", bufs=3, space="PSUM") as psO:

            # --- constants: one DMA ---
            cs = const.tile([128, CTOT], f32, tag="consts")
            nc.sync.dma_start(out=cs, in_=consts[:])
            wsb = {}
            for w, nm in enumerate(("q", "k", "v", "o")):
                for j in range(2):
                    base = CW + (w * 2 + j) * 256
                    wsb[nm, j] = cs[:, base:base + 256]
            tp_sb = cs[:, CTP:CTP + 12].rearrange("p (s j) -> p s j", s=S)
            bc_sb = cs[:, CBC:CBC + 8]
            er_sb = cs[0:4, CER:CER + 128]

            # attention probabilities accumulator [r, (b, s, j, t)]
            attn_acc = const.tile([4, BPC * S * 2 * S], f32, tag="attn_acc")
            nc.vector.memset(attn_acc, 0.0)
            # block-diagonal q (per j half): [p, r] nonzero only when p//32==r
            qblk = []
            for j in range(2):
                qb = const.tile([128, 4], f32, tag=f"qblk{j}")
                nc.vector.memset(qb, 0.0)
                qblk.append(qb)

            # settle all constant loads so per-frame instructions don't each
            # carry per-DMA-lane waits (ACT has a small sync-wait budget)
            tc.strict_bb_all_engine_barrier()

            for b in [bb % BPC for bb in range(BPC * repeat)]:
                ktc = cache.tile([128, 2, S], f32, tag="ktc")
                vtc = cache.tile([128, 2, S], f32, tag="vtc")
                for s in range(S):
                    slab = slabs.tile([128, 2, HWD], f32, tag="slab")
                    gsum = small.tile([128, 2], f32, tag="gsum")
                    for j in range(2):
                        nc.sync.dma_start(
                            out=slab[:, j, :],
                            in_=ff[b, s, j * 128:(j + 1) * 128, :])
                    if probe == "dma":
                        for j in range(2):
                            nc.scalar.dma_start(
                                out=cross[b, s, j * 128:(j + 1) * 128, :],
                                in_=slab[:, j, :])
                        continue
                    for j in range(2):
                        nc.vector.reduce_sum(
                            out=gsum[:, j:j + 1], in_=slab[:, j, :], axis=AX.X)
                    # g = gsum/4096 + temporal_pos[s]
                    g = small.tile([128, 2], f32, tag="g")
                    for j in range(2):
                        nc.scalar.activation(
                            out=g[:, j:j + 1], in_=gsum[:, j:j + 1],
                            func=AFT.Identity, bias=tp_sb[:, s, j:j + 1],
                            scale=1.0 / HWD)
                    # q/k/v projections -> one PSUM tile, col = w*2 + etile
                    qkvp = psA.tile([128, 6], f32)
                    for w, nm in enumerate(("q", "k", "v")):
                        for t in range(2):
                            col = w * 2 + t
                            nc.tensor.matmul(
                                qkvp[:, col:col + 1],
                                lhsT=wsb[nm, 0][:, t * 128:(t + 1) * 128],
                                rhs=g[:, 0:1], start=True, stop=False)
                            nc.tensor.matmul(
                                qkvp[:, col:col + 1],
                                lhsT=wsb[nm, 1][:, t * 128:(t + 1) * 128],
                                rhs=g[:, 1:2], start=False, stop=True)
                    # q (+bq) scattered into block-diag; k/v (+bias) into caches
                    for j in range(2):
                        for r in range(4):
                            pslc = slice(r * 32, (r + 1) * 32)
                            nc.scalar.activation(
                                out=qblk[j][pslc, r:r + 1], in_=qkvp[pslc, j:j + 1],
                                func=AFT.Identity, bias=bc_sb[pslc, j:j + 1])
                        nc.scalar.activation(
                            out=ktc[:, j, s:s + 1], in_=qkvp[:, 2 + j:3 + j],
                            func=AFT.Identity, bias=bc_sb[:, 2 + j:3 + j])
                        nc.scalar.activation(
                            out=vtc[:, j, s:s + 1], in_=qkvp[:, 4 + j:5 + j],
                            func=AFT.Identity, bias=bc_sb[:, 4 + j:5 + j])
                    # scores[r, j, t] = q_h . k_t  (h = j*4+r), causal: t <= s
                    # one PSUM tile: pe in cols 0:6, scores in cols 6:12
                    spe = psS.tile([128, 2, 2 * S], f32)
                    sc = spe[0:4, :, S:2 * S]
                    pe = spe[:, :, 0:S]
                    for j in range(2):
                        nc.tensor.matmul(
                            sc[:, j, 0:s + 1], lhsT=qblk[j],
                            rhs=ktc[:, j, 0:s + 1], start=True, stop=True)
                    # softmax over t (no max-subtraction: |score/sqrt(d)| < ~4);
                    # Exp's accum_out yields the per-head sum for free
                    pr = small.tile([4, 2, S], f32, tag="pr")
                    ssum = small.tile([4, 2], f32, tag="ssum")
                    for j in range(2):
                        nc.scalar.activation(
                            out=pr[:, j, 0:s + 1], in_=sc[:, j, 0:s + 1],
                            func=AFT.Exp, scale=SCALE,
                            accum_out=ssum[:, j:j + 1])
                    rcp = small.tile([4, 2], f32, tag="rcp")
                    nc.vector.reciprocal(out=rcp, in_=ssum)
                    for j in range(2):
                        base = ((b * S + s) * 2 + j) * S
                        nc.vector.tensor_scalar_mul(
                            out=attn_acc[:, base:base + s + 1],
                            in0=pr[:, j, 0:s + 1], scalar1=rcp[:, j:j + 1])
                    # expand probs to partition space, weight v cache, reduce
                    for j in range(2):
                        base = ((b * S + s) * 2 + j) * S
                        nc.tensor.matmul(
                            pe[:, j, 0:s + 1], lhsT=er_sb,
                            rhs=attn_acc[:, base:base + s + 1],
                            start=True, stop=True)
                    tmp = small.tile([128, 2, S], f32, tag="tmp")
                    nc.vector.tensor_mul(
                        out=tmp[:, :, 0:s + 1], in0=vtc[:, :, 0:s + 1],
                        in1=pe[:, :, 0:s + 1])
                    att = small.tile([128, 2], f32, tag="att")
                    nc.vector.reduce_sum(out=att, in_=tmp[:, :, 0:s + 1], axis=AX.X)
                    # output projection + bo
                    ogp = psO.tile([128, 2], f32)
                    for t in range(2):
                        nc.tensor.matmul(
                            ogp[:, t:t + 1], lhsT=wsb["o", 0][:, t * 128:(t + 1) * 128],
                            rhs=att[:, 0:1], start=True, stop=False)
                        nc.tensor.matmul(
                            ogp[:, t:t + 1], lhsT=wsb["o", 1][:, t * 128:(t + 1) * 128],
                            rhs=att[:, 1:2], start=False, stop=True)
                    ogs = small.tile([128, 2], f32, tag="ogs")
                    for j in range(2):
                        nc.scalar.activation(
                            out=ogs[:, j:j + 1], in_=ogp[:, j:j + 1],
                            func=AFT.Identity, bias=bc_sb[:, 6 + j:7 + j])
                    # broadcast-add onto the resident slab (both halves on
                    # ACT; DVE is saturated by the reduces), then store
                    for j in range(2):
                        nc.scalar.activation(
                            out=slab[:, j, :], in_=slab[:, j, :],
                            func=AFT.Identity, bias=ogs[:, j:j + 1])
                        nc.scalar.dma_start(
                            out=cross[b, s, j * 128:(j + 1) * 128, :],
                            in_=slab[:, j, :])

            # attention output: [4,(b,s,j,t)] -> attn[b, j*4+r, s, t]
            attn_view = attn_acc[:].rearrange(
                "r (b s j t) -> r b s j t", b=BPC, s=S, j=2)
            for b in range(BPC):
                for j in range(2):
                    nc.sync.dma_start(
                        out=attn[b, j * 4:(j + 1) * 4, :, :],
                        in_=attn_view[:, b, :, j, :])

    nc.finalize()
    return nc


def _get_nc(repeat=1, probe=None):
    key = ("nc", repeat, probe)
    if key not in _CACHE:
        _CACHE[key] = _build_nc(repeat, probe)
    return _CACHE[key]


def _prep_in_maps(inputs):
    ff = np.ascontiguousarray(
        np.asarray(inputs["frame_features"], dtype=np.float32)).reshape(B, S, E, HWD)
    blocks = []
    for key in ("Wq", "Wk", "Wv", "Wo"):
        wt = np.asarray(inputs[key], dtype=np.float32).T  # [f, e]
        blocks.append(wt[0:128, :])
        blocks.append(wt[128:256, :])
    tp = np.asarray(inputs["temporal_pos"], dtype=np.float32)
    blocks.append(tp.reshape(S, 2, 128).transpose(2, 0, 1).reshape(128, 12))
    bs = [np.asarray(inputs[k], dtype=np.float32).reshape(2, 128)
          for k in ("bq", "bk", "bv", "bo")]
    blocks.append(np.stack(bs, axis=0).transpose(2, 0, 1).reshape(128, 8))
    er = np.zeros((128, 128), dtype=np.float32)
    er[0:4] = np.repeat(np.eye(4, dtype=np.float32), 32, axis=1)
    blocks.append(er)
    consts = np.ascontiguousarray(np.concatenate(blocks, axis=1))
    assert consts.shape == (128, 2196), consts.shape

    in_maps = []
    for c in range(NCORES):
        m = {"consts": consts}
        m["ff"] = np.ascontiguousarray(ff[c * BPC:(c + 1) * BPC])
        in_maps.append(m)
    return in_maps


# ---------------------------------------------------------------------------
# Workaround: this container's walrus build accepts at most ONE sync-wait per
# instruction ("Too many sync wait commands" in setupSyncWait otherwise), but
# Tile's semaphore assignment attaches several.  Split extras onto single-wait
# NoOps inserted just before the instruction (same engine -> order preserved).
# ---------------------------------------------------------------------------

def _split_multiwaits(bir_json: bytes) -> bytes:
    import orjson

    js = orjson.loads(bir_json)
    for fn in js.get("functions", []):
        for blk in fn.get("blocks", []):
            out = []
            for inst in blk.get("instructions", []):
                si = inst.get("sync_info")
                waits = (si or {}).get("on_wait") or []
                if len(waits) > 1:
                    for k, w in enumerate(waits[:-1]):
                        out.append({
                            "engine": inst["engine"],
                            "ins": [],
                            "name": f"{inst['name']}-sw{k}",
                            "opcode": "NoOp",
                            "outs": [],
                            "sync_info": {"on_update": [], "on_wait": [w]},
                        })
                    si["on_wait"] = [waits[-1]]
                out.append(inst)
            blk["instructions"] = out
    return orjson.dumps(js)


def _install_wait_splitter():
    import concourse.bass_utils as bu
    import concourse.bass2jax as b2j

    if getattr(bu.compile_bir_kernel, "_split_wrapped", False):
        return
    orig = bu.compile_bir_kernel

    def wrapped(bir_json, tmpdir, neff_name="file.neff"):
        return orig(_split_multiwaits(bir_json), tmpdir, neff_name)

    wrapped._split_wrapped = True
    bu.compile_bir_kernel = wrapped
    b2j.compile_bir_kernel = wrapped


def _run(inputs):
    from concourse.bass_utils import run_bass_kernel_spmd

    _install_wait_splitter()

    # The axon NTFF-profile hook is not present in this container; make sure
    # run_bass_kernel_spmd never takes the trace path.
    os.environ["BASS_NEVER_TRACE"] = "1"
    nc = _get_nc()
    in_maps = _prep_in_maps(inputs)
    res = run_bass_kernel_spmd(
        nc, in_maps, core_ids=list(range(NCORES)), trace=False)
    cross = np.concatenate(
        [r["cross"] for r in res.results], axis=0).reshape(B, S, E, 64, 64)
    attn = np.concatenate([r["attn"] for r in res.results], axis=0)
    return cross, attn, res


def kernel(**inputs):
    cross, attn, _ = _run(inputs)
    return cross, attn


# ---------------------------------------------------------------------------
# Benchmark path (test.py only): cached jit, device-resident inputs, no
# donation (every output element is written by the kernel), so repeated
# executions measure pure device time.
# ---------------------------------------------------------------------------

def _get_exec(repeat=1, probe=None):
    key = ("exec", repeat, probe)
    if key in _CACHE:
        return _CACHE[key]
    import jax
    from concourse import bass2jax as b2j
    from concourse import mybir

    _install_wait_splitter()
    b2j.install_neuronx_cc_hook()
    nc = _get_nc(repeat, probe)
    partition_name = nc.partition_id_tensor.name if nc.partition_id_tensor else None

    in_names, out_names, out_avals = [], [], []
    for alloc in nc.m.functions[0].allocations:
        if not isinstance(alloc, mybir.MemoryLocationSet):
            continue
        name = alloc.memorylocations[0].name
        if alloc.kind == "ExternalInput":
            if name != partition_name:
                in_names.append(name)
        elif alloc.kind == "ExternalOutput":
            out_names.append(name)
            out_avals.append(jax.core.ShapedArray(
                tuple(alloc.tensor_shape), mybir.dt.np(alloc.dtype)))
    n_params = len(in_names)
    all_in = in_names + out_names
    if partition_name is not None:
        all_in = all_in + [partition_name]

    def _body(*args):
        operands = list(args)
        if partition_name is not None:
            operands.append(b2j.partition_id_tensor())
        return tuple(b2j._bass_exec_p.bind(
            *operands,
            out_avals=tuple(out_avals),
            in_names=tuple(all_in),
            out_names=tuple(out_names),
            lowering_input_output_aliases=(),
            sim_require_finite=True,
            sim_require_nnan=True,
            nc=nc,
        ))

    devices = jax.devices()[:NCORES]
    mesh = b2j.Mesh(np.asarray(devices), ("core",))
    P = b2j.PartitionSpec
    in_specs = (P("core"),) * (n_params + len(out_names))
    out_specs = (P("core"),) * len(out_names)
    fn = jax.jit(
        b2j.shard_map(_body, mesh=mesh, in_specs=in_specs,
                      out_specs=out_specs, check_rep=False),
        keep_unused=True)
    _CACHE[key] = (fn, mesh, in_names, out_names, out_avals, n_params)
    return _CACHE[key]


def _dev_args(inputs, repeat=1, probe=None):
    import jax
    import jax.numpy as jnp
    from jax.sharding import NamedSharding
    from jax.sharding import PartitionSpec as P

    fn, mesh, in_names, out_names, out_avals, n_params = _get_exec(repeat, probe)
    in_maps = _prep_in_maps(inputs)
    sh = NamedSharding(mesh, P("core"))
    dev_in = []
    for name in in_names:
        cat = np.concatenate(
            [np.asarray(in_maps[c][name]) for c in range(NCORES)], axis=0)
        dev_in.append(jax.device_put(cat, sh))
    zeros = jax.jit(
        lambda: tuple(
            jnp.zeros((NCORES * a.shape[0], *a.shape[1:]), a.dtype)
            for a in out_avals),
        out_shardings=tuple(sh for _ in out_avals))()
    return fn, out_names, (*dev_in, *zeros)


def _time_pipelined(fn, args, iters):
    import time as _time

    import jax

    jax.block_until_ready(fn(*args))
    best = float("inf")
    for _ in range(3):
        t0 = _time.perf_counter()
        last = None
        for _ in range(iters):
            last = fn(*args)
        jax.block_until_ready(last)
        best = min(best, (_time.perf_counter() - t0) / iters)
    return best


def bench_repeat(inputs, r_lo=8, r_hi=32, iters=12, probe=None):
    """Per-pass device time from the slope between R-repeat NEFF variants.

    The R-repeat NEFF runs the full computation R times back-to-back inside
    one device execution, so (T(r_hi) - T(r_lo)) / (r_hi - r_lo) cancels all
    dispatch/RPC overhead.
    """
    f_lo, out_names, args_lo = _dev_args(inputs, repeat=r_lo, probe=probe)
    f_hi, _, args_hi = _dev_args(inputs, repeat=r_hi, probe=probe)
    t_lo = _time_pipelined(f_lo, args_lo, iters)
    t_hi = _time_pipelined(f_hi, args_hi, iters)
    per_pass_ns = (t_hi - t_lo) * 1e9 / (r_hi - r_lo)

    import jax
    outs = f_lo(*args_lo)
    jax.block_until_ready(outs)
    cross = np.asarray(outs[out_names.index("cross")]).reshape(B, S, E, 64, 64)
    attn = np.asarray(outs[out_names.index("attn")]).reshape(B, NH, S, S)
    return cross, attn, per_pass_ns, (t_lo, t_hi)


def bench(inputs, iters=20, warmup=3):
    """Returns (cross, attn, per_iter_ns_pipelined, per_call_ns_list)."""
    import time as _time

    import jax
    import jax.numpy as jnp
    from jax.sharding import NamedSharding
    from jax.sharding import PartitionSpec as P

    fn, mesh, in_names, out_names, out_avals, n_params = _get_exec()
    in_maps = _prep_in_maps(inputs)
    sh = NamedSharding(mesh, P("core"))

    dev_in = []
    for i, name in enumerate(in_names):
        cat = np.concatenate(
            [np.asarray(in_maps[c][name]) for c in range(NCORES)], axis=0)
        dev_in.append(jax.device_put(cat, sh))
    zeros = jax.jit(
        lambda: tuple(
            jnp.zeros((NCORES * a.shape[0], *a.shape[1:]), a.dtype)
            for a in out_avals),
        out_shardings=tuple(sh for _ in out_avals))()
    args = (*dev_in, *zeros)

    outs = fn(*args)
    jax.block_until_ready(outs)

    per_call = []
    for _ in range(warmup):
        jax.block_until_ready(fn(*args))
    for _ in range(5):
        t0 = _time.perf_counter()
        jax.block_until_ready(fn(*args))
        per_call.append((_time.perf_counter() - t0) * 1e9)

    t0 = _time.perf_counter()
    last = None
    for _ in range(iters):
        last = fn(*args)
    jax.block_until_ready(last)
    pipelined_ns = (_time.perf_counter() - t0) * 1e9 / iters

    cross = np.asarray(outs[out_names.index("cross")]).reshape(
        B, S, E, 64, 64)
    attn = np.asarray(outs[out_names.index("attn")]).reshape(B, NH, S, S)
    return cross, attn, pipelined_ns, per_call


# revision 35
# speedup vs baseline: 4.1542x; 1.2093x over previous
"""CrossFrameAttention Trainium2 kernel.

Reference computation (B=16, S=6, E=256, heads=8, spatial 64x64):
  g    = mean_{h,w}(frame_features) + temporal_pos          [B,S,E]
  q/k/v= g @ W{q,k,v}.T + b                                 [B,S,E]
  attn = causal softmax(q k^T / sqrt(32)) per head          [B,8,S,S]
  out_g= (attn @ v) @ Wo.T + bo                             [B,S,E]
  cross= frame_features + out_g[..., None, None]
  returns (cross, attn)

Strategy: pure batch data-parallelism, 2 batches per core on 8 cores.
Single-pass streaming per (batch, frame): DMA the 4 MB frame slab into
SBUF, reduce it for the spatial mean, run the (tiny) incremental causal
attention row for that frame on-chip, broadcast-add out_g onto the slab
still resident in SBUF, DMA it out.  The 402 MB input is read exactly
once and written exactly once -> HBM roofline ~100 MB/core.

Channel layout on chip: e = j*128 + p (j in {0,1} selects the half,
p is the SBUF partition).  Head h = j*4 + p//32.

kernel() takes FULL inputs and returns the FULL (cross, attn) outputs.
"""

import math
import os

import numpy as np

B, S, E, HWD = 16, 6, 256, 4096
NH, HD = 8, 32
NCORES = 8
BPC = B // NCORES  # batches per core

# tuning knobs (overridable via env for experiments)
SLAB_BUFS = int(os.environ.get("SLAB_BUFS", "4"))
SMALL_BUFS = int(os.environ.get("SMALL_BUFS", "4"))
USE_ACCUM_EXP = os.environ.get("ACCUM_EXP", "0") == "1"

_CACHE = {}


def _build_nc(repeat=1, probe=None):
    import concourse.bass as bass
    import concourse.tile as tile
    from concourse import mybir

    f32 = mybir.dt.float32
    AFT = mybir.ActivationFunctionType
    AX = mybir.AxisListType

    nc = bass.Bass(trn_type="TRN2")

    ff = nc.dram_tensor("ff", [BPC, S, E, HWD], f32, kind="ExternalInput")
    # all small constants packed into one tensor -> one DMA -> one sem lane:
    # cols 0:2048   = W{q,k,v,o}.T halves, block (w*2+j)*256 : +256
    # cols 2048:2060 = temporal_pos cols (s*2+j)
    # cols 2060:2068 = biases cols (w*2+j)
    # cols 2068:2196 = head-expand matrix er on partitions 0:4
    CW, CTP, CBC, CER = 0, 2048, 2060, 2068
    CTOT = 2196
    consts = nc.dram_tensor("consts", [128, CTOT], f32, kind="ExternalInput")
    cross = nc.dram_tensor("cross", [BPC, S, E, HWD], f32, kind="ExternalOutput")
    attn = nc.dram_tensor("attn", [BPC, NH, S, S], f32, kind="ExternalOutput")

    SCALE = 1.0 / math.sqrt(HD)

    with tile.TileContext(nc) as tc:
        with tc.tile_pool(name="const", bufs=1) as const, \
             tc.tile_pool(name="slabs", bufs=SLAB_BUFS) as slabs, \
             tc.tile_pool(name="small", bufs=SMALL_BUFS) as small, \
             tc.tile_pool(name="cache", bufs=2) as cache, \
             tc.tile_pool(name="psA", bufs=2, space="PSUM") as psA, \
             tc.tile_pool(name="psS", bufs=2, space="PSUM") as psS, \
             tc.tile_pool(name="psE", bufs=2, space="PSUM") as psE, \
             tc.tile_pool(name="psO", bufs=2, space="PSUM") as psO:

            # --- constants: one DMA ---
            cs = const.tile([128, CTOT], f32, tag="consts")
            nc.sync.dma_start(out=cs, in_=consts[:])
            wsb = {}
            for w, nm in enumerate(("q", "k", "v", "o")):
                for j in range(2):
                    base = CW + (w * 2 + j) * 256
                    wsb[nm, j] = cs[:, base:base + 256]
            tp_sb = cs[:, CTP:CTP + 12].rearrange("p (s j) -> p s j", s=S)
            bc_sb = cs[:, CBC:CBC + 8]
            er_sb = cs[0:4, CER:CER + 128]

            # attention probabilities accumulator [r, (b, s, j, t)]
            attn_acc = const.tile([4, BPC * S * 2 * S], f32, tag="attn_acc")
            nc.vector.memset(attn_acc, 0.0)
            # block-diagonal q (per j half): [p, r] nonzero only when p//32==r
            qblk = []
            for j in range(2):
                qb = const.tile([128, 4], f32, tag=f"qblk{j}")
                nc.vector.memset(qb, 0.0)
                qblk.append(qb)

            # settle all constant loads so per-frame instructions don't each
            # carry per-DMA-lane waits (ACT has a small sync-wait budget)
            tc.strict_bb_all_engine_barrier()

            for b in [bb % BPC for bb in range(BPC * repeat)]:
                ktc = cache.tile([128, 2, S], f32, tag="ktc")
                vtc = cache.tile([128, 2, S], f32, tag="vtc")
                for s in range(S):
                    slab = slabs.tile([128, 2, HWD], f32, tag="slab")
                    gsum = small.tile([128, 2], f32, tag="gsum")
                    for j in range(2):
                        nc.sync.dma_start(
                            out=slab[:, j, :],
                            in_=ff[b, s, j * 128:(j + 1) * 128, :])
                    if probe == "dma":
                        for j in range(2):
                            nc.scalar.dma_start(
                                out=cross[b, s, j * 128:(j + 1) * 128, :],
                                in_=slab[:, j, :])
                        continue
                    for j in range(2):
                        nc.vector.reduce_sum(
                            out=gsum[:, j:j + 1], in_=slab[:, j, :], axis=AX.X)
                    # g = gsum/4096 + temporal_pos[s]
                    g = small.tile([128, 2], f32, tag="g")
                    for j in range(2):
                        nc.scalar.activation(
                            out=g[:, j:j + 1], in_=gsum[:, j:j + 1],
                            func=AFT.Identity, bias=tp_sb[:, s, j:j + 1],
                            scale=1.0 / HWD)
                    # q/k/v projections -> one PSUM tile, col = w*2 + etile
                    qkvp = psA.tile([128, 6], f32)
                    for w, nm in enumerate(("q", "k", "v")):
                        for t in range(2):
                            col = w * 2 + t
                            nc.tensor.matmul(
                                qkvp[:, col:col + 1],
                                lhsT=wsb[nm, 0][:, t * 128:(t + 1) * 128],
                                rhs=g[:, 0:1], start=True, stop=False)
                            nc.tensor.matmul(
                                qkvp[:, col:col + 1],
                                lhsT=wsb[nm, 1][:, t * 128:(t + 1) * 128],
                                rhs=g[:, 1:2], start=False, stop=True)
                    # q (+bq) scattered into block-diag; k/v (+bias) into caches
                    for j in range(2):
                        for r in range(4):
                            pslc = slice(r * 32, (r + 1) * 32)
                            nc.scalar.activation(
                                out=qblk[j][pslc, r:r + 1], in_=qkvp[pslc, j:j + 1],
                                func=AFT.Identity, bias=bc_sb[pslc, j:j + 1])
                        nc.scalar.activation(
                            out=ktc[:, j, s:s + 1], in_=qkvp[:, 2 + j:3 + j],
                            func=AFT.Identity, bias=bc_sb[:, 2 + j:3 + j])
                        nc.scalar.activation(
                            out=vtc[:, j, s:s + 1], in_=qkvp[:, 4 + j:5 + j],
                            func=AFT.Identity, bias=bc_sb[:, 4 + j:5 + j])
                    # scores[r, j, t] = q_h . k_t  (h = j*4+r), causal: t <= s
                    sc = psS.tile([4, 2, S], f32)
                    for j in range(2):
                        nc.tensor.matmul(
                            sc[:, j, 0:s + 1], lhsT=qblk[j],
                            rhs=ktc[:, j, 0:s + 1], start=True, stop=True)
                    # softmax over t (no max-subtraction: |score/sqrt(d)| < ~4);
                    # Exp's accum_out yields the per-head sum for free
                    pr = small.tile([4, 2, S], f32, tag="pr")
                    ssum = small.tile([4, 2], f32, tag="ssum")
                    if USE_ACCUM_EXP:
                        for j in range(2):
                            nc.scalar.activation(
                                out=pr[:, j, 0:s + 1], in_=sc[:, j, 0:s + 1],
                                func=AFT.Exp, scale=SCALE,
                                accum_out=ssum[:, j:j + 1])
                    else:
                        nc.scalar.activation(
                            out=pr[:, :, 0:s + 1], in_=sc[:, :, 0:s + 1],
                            func=AFT.Exp, scale=SCALE)
                        nc.vector.reduce_sum(
                            out=ssum, in_=pr[:, :, 0:s + 1], axis=AX.X)
                    rcp = small.tile([4, 2], f32, tag="rcp")
                    nc.vector.reciprocal(out=rcp, in_=ssum)
                    for j in range(2):
                        base = ((b * S + s) * 2 + j) * S
                        nc.vector.tensor_scalar_mul(
                            out=attn_acc[:, base:base + s + 1],
                            in0=pr[:, j, 0:s + 1], scalar1=rcp[:, j:j + 1])
                    # expand probs to partition space, weight v cache, reduce
                    pe = psE.tile([128, 2, S], f32)
                    for j in range(2):
                        base = ((b * S + s) * 2 + j) * S
                        nc.tensor.matmul(
                            pe[:, j, 0:s + 1], lhsT=er_sb,
                            rhs=attn_acc[:, base:base + s + 1],
                            start=True, stop=True)
                    tmp = small.tile([128, 2, S], f32, tag="tmp")
                    nc.vector.tensor_mul(
                        out=tmp[:, :, 0:s + 1], in0=vtc[:, :, 0:s + 1],
                        in1=pe[:, :, 0:s + 1])
                    att = small.tile([128, 2], f32, tag="att")
                    nc.vector.reduce_sum(out=att, in_=tmp[:, :, 0:s + 1], axis=AX.X)
                    # output projection + bo
                    ogp = psO.tile([128, 2], f32)
                    for t in range(2):
                        nc.tensor.matmul(
                            ogp[:, t:t + 1], lhsT=wsb["o", 0][:, t * 128:(t + 1) * 128],
                            rhs=att[:, 0:1], start=True, stop=False)
                        nc.tensor.matmul(
                            ogp[:, t:t + 1], lhsT=wsb["o", 1][:, t * 128:(t + 1) * 128],
                            rhs=att[:, 1:2], start=False, stop=True)
                    ogs = small.tile([128, 2], f32, tag="ogs")
                    for j in range(2):
                        nc.scalar.activation(
                            out=ogs[:, j:j + 1], in_=ogp[:, j:j + 1],
                            func=AFT.Identity, bias=bc_sb[:, 6 + j:7 + j])
                    # broadcast-add onto the resident slab (both halves on
                    # ACT; DVE is saturated by the reduces), then store
                    for j in range(2):
                        nc.scalar.activation(
                            out=slab[:, j, :], in_=slab[:, j, :],
                            func=AFT.Identity, bias=ogs[:, j:j + 1])
                        nc.scalar.dma_start(
                            out=cross[b, s, j * 128:(j + 1) * 128, :],
                            in_=slab[:, j, :])

            # attention output: [4,(b,s,j,t)] -> attn[b, j*4+r, s, t]
            attn_view = attn_acc[:].rearrange(
                "r (b s j t) -> r b s j t", b=BPC, s=S, j=2)
            for b in range(BPC):
                for j in range(2):
                    nc.sync.dma_start(
                        out=attn[b, j * 4:(j + 1) * 4, :, :],
                        in_=attn_view[:, b, :, j, :])

    nc.finalize()
    return nc


def _get_nc(repeat=1, probe=None):
    key = ("nc", repeat, probe)
    if key not in _CACHE:
        _CACHE[key] = _build_nc(repeat, probe)
    return _CACHE[key]


def _prep_in_maps(inputs):
    ff = np.ascontiguousarray(
        np.asarray(inputs["frame_features"], dtype=np.float32)).reshape(B, S, E, HWD)
    blocks = []
    for key in ("Wq", "Wk", "Wv", "Wo"):
        wt = np.asarray(inputs[key], dtype=np.float32).T  # [f, e]
        blocks.append(wt[0:128, :])
        blocks.append(wt[128:256, :])
    tp = np.asarray(inputs["temporal_pos"], dtype=np.float32)
    blocks.append(tp.reshape(S, 2, 128).transpose(2, 0, 1).reshape(128, 12))
    bs = [np.asarray(inputs[k], dtype=np.float32).reshape(2, 128)
          for k in ("bq", "bk", "bv", "bo")]
    blocks.append(np.stack(bs, axis=0).transpose(2, 0, 1).reshape(128, 8))
    er = np.zeros((128, 128), dtype=np.float32)
    er[0:4] = np.repeat(np.eye(4, dtype=np.float32), 32, axis=1)
    blocks.append(er)
    consts = np.ascontiguousarray(np.concatenate(blocks, axis=1))
    assert consts.shape == (128, 2196), consts.shape

    in_maps = []
    for c in range(NCORES):
        m = {"consts": consts}
        m["ff"] = np.ascontiguousarray(ff[c * BPC:(c + 1) * BPC])
        in_maps.append(m)
    return in_maps


# ---------------------------------------------------------------------------
# Workaround: this container's walrus build accepts at most ONE sync-wait per
# instruction ("Too many sync wait commands" in setupSyncWait otherwise), but
# Tile's semaphore assignment attaches several.  Split extras onto single-wait
# NoOps inserted just before the instruction (same engine -> order preserved).
# ---------------------------------------------------------------------------

def _split_multiwaits(bir_json: bytes) -> bytes:
    import orjson

    js = orjson.loads(bir_json)
    for fn in js.get("functions", []):
        for blk in fn.get("blocks", []):
            out = []
            for inst in blk.get("instructions", []):
                si = inst.get("sync_info")
                waits = (si or {}).get("on_wait") or []
                if len(waits) > 1:
                    for k, w in enumerate(waits[:-1]):
                        out.append({
                            "engine": inst["engine"],
                            "ins": [],
                            "name": f"{inst['name']}-sw{k}",
                            "opcode": "NoOp",
                            "outs": [],
                            "sync_info": {"on_update": [], "on_wait": [w]},
                        })
                    si["on_wait"] = [waits[-1]]
                out.append(inst)
            blk["instructions"] = out
    return orjson.dumps(js)


def _install_wait_splitter():
    import concourse.bass_utils as bu
    import concourse.bass2jax as b2j

    if getattr(bu.compile_bir_kernel, "_split_wrapped", False):
        return
    orig = bu.compile_bir_kernel

    def wrapped(bir_json, tmpdir, neff_name="file.neff"):
        return orig(_split_multiwaits(bir_json), tmpdir, neff_name)

    wrapped._split_wrapped = True
    bu.compile_bir_kernel = wrapped
    b2j.compile_bir_kernel = wrapped


def _run(inputs):
    from concourse.bass_utils import run_bass_kernel_spmd

    _install_wait_splitter()

    # The axon NTFF-profile hook is not present in this container; make sure
    # run_bass_kernel_spmd never takes the trace path.
    os.environ["BASS_NEVER_TRACE"] = "1"
    nc = _get_nc()
    in_maps = _prep_in_maps(inputs)
    res = run_bass_kernel_spmd(
        nc, in_maps, core_ids=list(range(NCORES)), trace=False)
    cross = np.concatenate(
        [r["cross"] for r in res.results], axis=0).reshape(B, S, E, 64, 64)
    attn = np.concatenate([r["attn"] for r in res.results], axis=0)
    return cross, attn, res


def kernel(**inputs):
    cross, attn, _ = _run(inputs)
    return cross, attn


# ---------------------------------------------------------------------------
# Benchmark path (test.py only): cached jit, device-resident inputs, no
# donation (every output element is written by the kernel), so repeated
# executions measure pure device time.
# ---------------------------------------------------------------------------

def _get_exec(repeat=1, probe=None):
    key = ("exec", repeat, probe)
    if key in _CACHE:
        return _CACHE[key]
    import jax
    from concourse import bass2jax as b2j
    from concourse import mybir

    _install_wait_splitter()
    b2j.install_neuronx_cc_hook()
    nc = _get_nc(repeat, probe)
    partition_name = nc.partition_id_tensor.name if nc.partition_id_tensor else None

    in_names, out_names, out_avals = [], [], []
    for alloc in nc.m.functions[0].allocations:
        if not isinstance(alloc, mybir.MemoryLocationSet):
            continue
        name = alloc.memorylocations[0].name
        if alloc.kind == "ExternalInput":
            if name != partition_name:
                in_names.append(name)
        elif alloc.kind == "ExternalOutput":
            out_names.append(name)
            out_avals.append(jax.core.ShapedArray(
                tuple(alloc.tensor_shape), mybir.dt.np(alloc.dtype)))
    n_params = len(in_names)
    all_in = in_names + out_names
    if partition_name is not None:
        all_in = all_in + [partition_name]

    def _body(*args):
        operands = list(args)
        if partition_name is not None:
            operands.append(b2j.partition_id_tensor())
        return tuple(b2j._bass_exec_p.bind(
            *operands,
            out_avals=tuple(out_avals),
            in_names=tuple(all_in),
            out_names=tuple(out_names),
            lowering_input_output_aliases=(),
            sim_require_finite=True,
            sim_require_nnan=True,
            nc=nc,
        ))

    devices = jax.devices()[:NCORES]
    mesh = b2j.Mesh(np.asarray(devices), ("core",))
    P = b2j.PartitionSpec
    in_specs = (P("core"),) * (n_params + len(out_names))
    out_specs = (P("core"),) * len(out_names)
    fn = jax.jit(
        b2j.shard_map(_body, mesh=mesh, in_specs=in_specs,
                      out_specs=out_specs, check_rep=False),
        keep_unused=True)
    _CACHE[key] = (fn, mesh, in_names, out_names, out_avals, n_params)
    return _CACHE[key]


def _dev_args(inputs, repeat=1, probe=None):
    import jax
    import jax.numpy as jnp
    from jax.sharding import NamedSharding
    from jax.sharding import PartitionSpec as P

    fn, mesh, in_names, out_names, out_avals, n_params = _get_exec(repeat, probe)
    in_maps = _prep_in_maps(inputs)
    sh = NamedSharding(mesh, P("core"))
    dev_in = []
    for name in in_names:
        cat = np.concatenate(
            [np.asarray(in_maps[c][name]) for c in range(NCORES)], axis=0)
        dev_in.append(jax.device_put(cat, sh))
    zeros = jax.jit(
        lambda: tuple(
            jnp.zeros((NCORES * a.shape[0], *a.shape[1:]), a.dtype)
            for a in out_avals),
        out_shardings=tuple(sh for _ in out_avals))()
    return fn, out_names, (*dev_in, *zeros)


def _time_pipelined(fn, args, iters):
    import time as _time

    import jax

    jax.block_until_ready(fn(*args))
    best = float("inf")
    for _ in range(3):
        t0 = _time.perf_counter()
        last = None
        for _ in range(iters):
            last = fn(*args)
        jax.block_until_ready(last)
        best = min(best, (_time.perf_counter() - t0) / iters)
    return best


def bench_repeat(inputs, r_lo=8, r_hi=32, iters=12, probe=None):
    """Per-pass device time from the slope between R-repeat NEFF variants.

    The R-repeat NEFF runs the full computation R times back-to-back inside
    one device execution, so (T(r_hi) - T(r_lo)) / (r_hi - r_lo) cancels all
    dispatch/RPC overhead.
    """
    f_lo, out_names, args_lo = _dev_args(inputs, repeat=r_lo, probe=probe)
    f_hi, _, args_hi = _dev_args(inputs, repeat=r_hi, probe=probe)
    t_lo = _time_pipelined(f_lo, args_lo, iters)
    t_hi = _time_pipelined(f_hi, args_hi, iters)
    per_pass_ns = (t_hi - t_lo) * 1e9 / (r_hi - r_lo)

    import jax
    outs = f_lo(*args_lo)
    jax.block_until_ready(outs)
    cross = np.asarray(outs[out_names.index("cross")]).reshape(B, S, E, 64, 64)
    attn = np.asarray(outs[out_names.index("attn")]).reshape(B, NH, S, S)
    return cross, attn, per_pass_ns, (t_lo, t_hi)


def bench(inputs, iters=20, warmup=3):
    """Returns (cross, attn, per_iter_ns_pipelined, per_call_ns_list)."""
    import time as _time

    import jax
    import jax.numpy as jnp
    from jax.sharding import NamedSharding
    from jax.sharding import PartitionSpec as P

    fn, mesh, in_names, out_names, out_avals, n_params = _get_exec()
    in_maps = _prep_in_maps(inputs)
    sh = NamedSharding(mesh, P("core"))

    dev_in = []
    for i, name in enumerate(in_names):
        cat = np.concatenate(
            [np.asarray(in_maps[c][name]) for c in range(NCORES)], axis=0)
        dev_in.append(jax.device_put(cat, sh))
    zeros = jax.jit(
        lambda: tuple(
            jnp.zeros((NCORES * a.shape[0], *a.shape[1:]), a.dtype)
            for a in out_avals),
        out_shardings=tuple(sh for _ in out_avals))()
    args = (*dev_in, *zeros)

    outs = fn(*args)
    jax.block_until_ready(outs)

    per_call = []
    for _ in range(warmup):
        jax.block_until_ready(fn(*args))
    for _ in range(5):
        t0 = _time.perf_counter()
        jax.block_until_ready(fn(*args))
        per_call.append((_time.perf_counter() - t0) * 1e9)

    t0 = _time.perf_counter()
    last = None
    for _ in range(iters):
        last = fn(*args)
    jax.block_until_ready(last)
    pipelined_ns = (_time.perf_counter() - t0) * 1e9 / iters

    cross = np.asarray(outs[out_names.index("cross")]).reshape(
        B, S, E, 64, 64)
    attn = np.asarray(outs[out_names.index("attn")]).reshape(B, NH, S, S)
    return cross, attn, pipelined_ns, per_call
